# revision 56
# baseline (speedup 1.0000x reference)
"""Fused single-launch BPCA pooling: bf16 gram + on-device top-eigenvector
(shifted power iteration by repeated squaring) + bf16 projection.

v6: lag-2 software pipeline.  body(b) = loads(b) | proj(b-2) |
extraction+eig(b-1) | casts(b) | gram(b).  All proj/plane inputs are
ready at body start (v_rep computed a full period earlier), the gram is
bf16 (fp32 sustained matmul streams throttle ~2x), and the eigensolve
is a DVE-only chain: L-inf shift (no sqrt -> no ACT round trip), one
1/||C||^2 renorm, host-supplied mu-outer-product and -mu, accum_out
fused reductions, stats written by direct DMA.

Host fixes sign/scale of the output using the returned S and
unnormalized v_dev (jax-cpu eigh for the reference LAPACK sign
convention).
"""

import numpy as np
from contextlib import ExitStack

import concourse.bass as bass
import concourse.tile as tile
from concourse import bacc, mybir
from concourse.bass_utils import run_bass_kernel_spmd

B, H, W, C = 32, 64, 64, 512
N_CORES = 8
BPC = B // N_CORES
SAMPLE = H * W * C
NROWS = SAMPLE // 4
OUT_SAMPLE = SAMPLE // 4
F32 = mybir.dt.float32
BF16 = mybir.dt.bfloat16
I32 = mybir.dt.int32
ALU = mybir.AluOpType
AF = mybir.ActivationFunctionType
AXL = mybir.AxisListType

NSQ = 7                       # squarings; top-eig contamination ~ratio^-128
EVEC = [0.9129, -0.6011, 0.3683, 1.0577]   # fixed generic seed vector
AUXW = 24 * BPC + 8


def _in_dram_ap(x, b, half, q, h2):
    off = b * SAMPLE + half * 32768 + q * 4096 + h2 * 2048
    return bass.AP(x, off, [[65536, 32], [8192, 4], [1, 2048]])


def _v(ap, axes, extra_off=0):
    """Free-dim view of a [128, F] (or [P, F]) tile AP with custom free axes."""
    return bass.AP(ap.tensor, ap.offset + extra_off, [list(ap.ap[0])] + axes)


def _build_fused():
    nc = bacc.Bacc("TRN2", target_bir_lowering=False, debug=False)
    x = nc.dram_tensor("x", [BPC * SAMPLE], F32, kind="ExternalInput")
    aux = nc.dram_tensor("aux", [128, AUXW], F32, kind="ExternalInput")
    y = nc.dram_tensor("y", [BPC * OUT_SAMPLE], BF16, kind="ExternalOutput")
    st = nc.dram_tensor("stats", [BPC, 20], F32, kind="ExternalOutput")

    with tile.TileContext(nc) as tc, ExitStack() as ctx:
        const = ctx.enter_context(tc.tile_pool(name="const", bufs=1))
        stage = ctx.enter_context(tc.tile_pool(name="stage", bufs=10))
        chunks = ctx.enter_context(tc.tile_pool(name="chunks", bufs=12))
        psum = ctx.enter_context(tc.tile_pool(name="psum", bufs=2, space="PSUM"))
        psx = ctx.enter_context(tc.tile_pool(name="psx", bufs=1, space="PSUM"))
        red = ctx.enter_context(tc.tile_pool(name="red", bufs=2))
        eig = ctx.enter_context(tc.tile_pool(name="eig", bufs=2))
        stag = ctx.enter_context(tc.tile_pool(name="stag", bufs=1))
        wpool = ctx.enter_context(tc.tile_pool(name="wpool", bufs=2))
        psv = ctx.enter_context(tc.tile_pool(name="psv", bufs=4, space="PSUM"))

        # ---------- constants ----------
        auxp = const.tile([128, AUXW], F32)
        nc.sync.dma_start(auxp[:], bass.AP(aux, 0, [[AUXW, 128], [1, AUXW]]))
        pidx_i = const.tile([128, 1], I32)
        nc.gpsimd.iota(pidx_i[:], [[0, 1]], base=0, channel_multiplier=1)
        pmod_i = const.tile([128, 1], I32)
        nc.vector.tensor_scalar(pmod_i[:], pidx_i[:], 3, None, ALU.bitwise_and)
        pgrp_i = const.tile([128, 1], I32)
        nc.vector.tensor_scalar(pgrp_i[:], pidx_i[:], -4, None, ALU.bitwise_and)
        E4_i = const.tile([128, 4], I32)
        for k in range(4):
            nc.vector.tensor_scalar(E4_i[:, k:k + 1], pmod_i[:], k, None, ALU.is_equal)
        E4 = const.tile([128, 4], F32)
        nc.vector.tensor_copy(E4[:], E4_i[:])
        # mask[p, n] = (n//4 == p//4), [128, 128]
        cidx_i = const.tile([128, 128], I32)
        nc.gpsimd.iota(cidx_i[:], [[1, 128]], base=0, channel_multiplier=0)
        cgrp_i = const.tile([128, 128], I32)
        nc.vector.tensor_scalar(cgrp_i[:], cidx_i[:], -4, None, ALU.bitwise_and)
        cgrp = const.tile([128, 128], F32)
        nc.vector.tensor_copy(cgrp[:], cgrp_i[:])
        pgrp = const.tile([128, 1], F32)
        nc.vector.tensor_copy(pgrp[:], pgrp_i[:])
        mask = const.tile([128, 128], F32)
        nc.vector.tensor_scalar(mask[:], cgrp[:], pgrp[:], 0.0, ALU.subtract, ALU.is_equal)
        # FM[p, u] = (u//4 == p), u in [0,16)  (only partitions 0..3 used)
        uidx_i = const.tile([128, 16], I32)
        nc.gpsimd.iota(uidx_i[:], [[1, 16]], base=0, channel_multiplier=0)
        ugrp_i = const.tile([128, 16], I32)
        nc.vector.tensor_scalar(ugrp_i[:], uidx_i[:], -4, None, ALU.bitwise_and)
        ugrp = const.tile([128, 16], F32)   # 4*(u//4)
        nc.vector.tensor_copy(ugrp[:], ugrp_i[:])
        pidx4 = const.tile([128, 1], F32)   # p*4
        nc.vector.tensor_copy(pidx4[:], pidx_i[:])
        nc.vector.tensor_scalar(pidx4[:], pidx4[:], 4.0, None, ALU.mult)
        FM = const.tile([128, 16], F32)
        nc.vector.tensor_scalar(FM[:], ugrp[:], pidx4[:], 0.0, ALU.subtract, ALU.is_equal)
        # dm16[p, u] = (u//4 == u%4): flat identity; dm16qn = -identity/4
        umod_i = const.tile([128, 16], I32)
        nc.vector.tensor_scalar(umod_i[:], uidx_i[:], 3, None, ALU.bitwise_and)
        ud_i = const.tile([128, 16], I32)
        nc.vector.tensor_scalar(ud_i[:], ugrp_i[:], 2, None, ALU.arith_shift_right)
        umod = const.tile([128, 16], F32)
        nc.vector.tensor_copy(umod[:], umod_i[:])
        ud = const.tile([128, 16], F32)
        nc.vector.tensor_copy(ud[:], ud_i[:])
        dm16 = const.tile([128, 16], F32)
        nc.vector.tensor_tensor(dm16[:], ud[:], umod[:], ALU.is_equal)
        dm16qn = const.tile([128, 16], F32)
        nc.vector.tensor_scalar(dm16qn[:], dm16[:], -0.25, None, ALU.mult)
        ones4x128 = const.tile([4, 128], F32)
        nc.vector.memset(ones4x128[:], 1.0)
        # full [128,128] identity in bf16 (stationary base for diag(v_k))
        pidf = const.tile([128, 1], F32)
        nc.vector.tensor_copy(pidf[:], pidx_i[:])
        cidf = const.tile([128, 128], F32)
        nc.vector.tensor_copy(cidf[:], cidx_i[:])
        idf = const.tile([128, 128], F32)
        nc.vector.tensor_scalar(idf[:], cidf[:], pidf[:], 0.0, ALU.subtract, ALU.is_equal)
        idb = const.tile([128, 128], BF16)
        nc.vector.tensor_copy(idb[:], idf[:])

        def emit_extraction(psg):
            m = red.tile([128, 128], F32, tag="m")
            nc.vector.tensor_mul(m[:], psg[:], mask[:])
            psE = psx.tile([4, 128], F32, tag="psE")
            nc.tensor.matmul(psE[:], E4[:], m[:], start=True, stop=True)
            # S44[k, l] = sum_c psE[k, 4c+l]: strided-innermost reduce from PSUM
            S44 = red.tile([4, 4], F32, tag="S44")
            nc.vector.tensor_reduce(S44[:], _v(psE[:], [[1, 4], [4, 32]]),
                                    AXL.X, ALU.add)
            # spread S [4,4] -> [4,16] rows: Fm16[p,(j,l)] = S[p,l] * (p==j)
            Fm16 = red.tile([4, 16], F32, tag="Fm16")
            s_b = _v(S44[:], [[0, 4], [1, 4]])       # [4, j(bcast), l]
            nc.vector.tensor_tensor(Fm16[:].rearrange("p (j l) -> p j l", j=4),
                                    s_b, FM[0:4, :].rearrange("p (j l) -> p j l", j=4),
                                    ALU.mult)
            psS = psx.tile([128, 16], F32, tag="psS")
            nc.tensor.matmul(psS[:], ones4x128[:], Fm16[:], start=True, stop=True)
            return psS

        def emit_eig(b, psS):
            base = 24 * b
            negmu = auxp[:, base:base + 4]
            mmf = auxp[:, base + 4:base + 20]
            evec = auxp[:, 24 * BPC:24 * BPC + 4]
            covf = eig.tile([128, 16], F32, tag="covf")
            nc.vector.scalar_tensor_tensor(covf[:], psS[:], 1.0 / NROWS, mmf,
                                           ALU.mult, ALU.subtract)
            # -tr/4 via accum over covf * (-I/4)
            dgj = eig.tile([128, 16], F32, tag="dgj")
            trqn = eig.tile([128, 1], F32, tag="trqn")
            nc.vector.scalar_tensor_tensor(dgj[:], covf[:], 1.0, dm16qn[:],
                                           ALU.mult, ALU.mult, accum_out=trqn[:])
            B0 = eig.tile([128, 16], F32, tag="B0")
            nc.vector.scalar_tensor_tensor(B0[:], dm16[:], trqn[:], covf[:],
                                           ALU.mult, ALU.add)
            # shift s = 2*max|B0| (>= spectral bound margin for traceless 4x4)
            mx = eig.tile([128, 1], F32, tag="mx")
            nc.vector.tensor_reduce(mx[:], B0[:], AXL.X, ALU.max)
            mn = eig.tile([128, 1], F32, tag="mn")
            nc.vector.tensor_reduce(mn[:], B0[:], AXL.X, ALU.min)
            nc.vector.tensor_scalar(mx[:], mx[:], 2.0, None, ALU.mult)
            nc.vector.tensor_scalar(mn[:], mn[:], -2.0, None, ALU.mult)
            sft = eig.tile([128, 1], F32, tag="sft")
            nc.vector.tensor_tensor(sft[:], mx[:], mn[:], ALU.max)
            Bc = eig.tile([128, 16], F32, tag="Bc")
            nc.vector.scalar_tensor_tensor(Bc[:], dm16[:], sft[:], B0[:],
                                           ALU.mult, ALU.add)  # B0 + s I
            srec = eig.tile([128, 1], F32, tag="srec")
            nc.vector.reciprocal(srec[:], sft[:])
            nc.vector.tensor_scalar(Bc[:], Bc[:], srec[:], 0.5, ALU.mult, ALU.mult)
            # squarings, ping-pong; single renorm by 1/||C||_F^2 at it==2
            prod = eig.tile([128, 64], F32, tag="prod")
            Cc = eig.tile([128, 16], F32, tag="Cc")
            sqt = eig.tile([128, 16], F32, tag="sqt")
            cur, nxt = Bc, Cc
            for it in range(NSQ):
                b_ik = _v(cur[:], [[4, 4], [0, 4], [1, 4]])  # [p,i,j,k]=B[4i+k]
                b_kj = _v(cur[:], [[0, 4], [1, 4], [4, 4]])  # [p,i,j,k]=B[4k+j]
                nc.vector.tensor_tensor(
                    prod[:].rearrange("p (i j k) -> p i j k", i=4, j=4),
                    b_ik, b_kj, ALU.mult)
                nc.vector.tensor_reduce(
                    nxt[:].rearrange("p (i j) -> p i j", i=4),
                    prod[:].rearrange("p (i j k) -> p i j k", i=4, j=4),
                    AXL.X, ALU.add)
                if it == 2:
                    nrm2 = eig.tile([128, 1], F32, tag="nrm2")
                    nc.vector.scalar_tensor_tensor(
                        sqt[:], nxt[:], 1.0, nxt[:], ALU.mult, ALU.mult,
                        accum_out=nrm2[:])
                    nc.vector.reciprocal(nrm2[:], nrm2[:])
                    nc.vector.tensor_scalar(nxt[:], nxt[:], nrm2[:], None, ALU.mult)
                cur, nxt = nxt, cur
            # v = B @ e  (replicated): v_rep[p, i] = sum_j B[4i+j] e[j]
            vprod = eig.tile([128, 16], F32, tag="vprod")
            nc.vector.tensor_tensor(
                vprod[:].rearrange("p (i j) -> p i j", i=4),
                _v(cur[:], [[4, 4], [1, 4]]), _v(evec, [[0, 4], [1, 4]]), ALU.mult)
            v_rep = eig.tile([128, 4], F32, tag="v_rep")
            nc.vector.tensor_reduce(
                v_rep[:].rearrange("p (i u) -> p i u", i=4),
                vprod[:].rearrange("p (i j) -> p i j", i=4), AXL.X, ALU.add)
            # negc = v . (-mu) via accum
            mvp = eig.tile([128, 4], F32, tag="mvp")
            negc = eig.tile([128, 1], F32, tag="negc")
            nc.vector.scalar_tensor_tensor(mvp[:], v_rep[:], 1.0, negmu,
                                           ALU.mult, ALU.mult, accum_out=negc[:])
            # stats out: [1, 20] = Sflat | v_dev (DMA issued from gpsimd so
            # the sync queue's load triggers never wait on the eig chain)
            stt = eig.tile([1, 20], F32, tag="stt")
            nc.vector.tensor_copy(stt[:, 0:16], psS[0:1, :])
            nc.vector.tensor_copy(stt[:, 16:20], v_rep[0:1, :])
            nc.gpsimd.dma_start(bass.AP(st, b * 20, [[20, 1], [1, 20]]), stt[:])
            # stationary diag(v_k) bf16 matrices for the PE projection
            W4 = []
            for k in range(4):
                Wk = wpool.tile([128, 128], BF16, tag=f"W{k}", name=f"W_{b}_{k}")
                nc.vector.tensor_scalar(Wk[:], idb[:], v_rep[:, k:k + 1], None,
                                        ALU.mult)
                W4.append(Wk)
            return v_rep, negc, W4

        def emit_proj(pb, pctiles, W4, pnegc):
            # PE projection: per PSUM bank, 4 accumulating matmuls with
            # stationary diag(v_k) and moving = strided channel-k view;
            # one ACT op applies -c and writes bf16 to stg; gpsimd DMAs out.
            stg = stag.tile([128, 4096], BF16, tag="stg", name=f"stg_{pb}")
            for ci, t in enumerate(pctiles):
                half, q = divmod(ci, 2)
                for hf in range(2):
                    ps = psv.tile([128, 512], F32, tag="sv",
                                  name=f"sv_{pb}_{ci}_{hf}")
                    for k in range(4):
                        rhs = bass.AP(t[:].tensor,
                                      t[:].offset + hf * 2048 + k,
                                      [list(t[:].ap[0])] + [[4, 512]])
                        nc.tensor.matmul(ps[:], W4[k][:], rhs,
                                         start=(k == 0), stop=(k == 3))
                    nc.scalar.activation(
                        stg[:, ci * 1024 + hf * 512:ci * 1024 + (hf + 1) * 512],
                        ps[:], AF.Identity, bias=pnegc[:], scale=1.0)
                nc.gpsimd.dma_start(
                    bass.AP(y, pb * OUT_SAMPLE + q * 2048 + half * 256,
                            [[4096, 128], [512, 4], [1, 256]]),
                    _v(stg[:], [[256, 4], [1, 256]], extra_off=ci * 1024))

        # Lag-2 pipeline.
        states = {}
        vstate = {}
        for b in range(BPC):
            # ---------- loads: 8 half-chunk staging tiles; triggers split
            # across two queues to halve head-of-line blocking ----------
            ftiles = []
            for h in range(8):
                ci, h2 = divmod(h, 2)
                half, q = divmod(ci, 2)
                tf = stage.tile([128, 2048], F32, tag="stagef", name=f"tf_{b}_{h}")
                eng = nc.sync if h % 2 == 0 else nc.gpsimd
                eng.dma_start(tf[:], _in_dram_ap(x, b, half, q, h2))
                ftiles.append(tf)
            ctiles = [
                chunks.tile([128, 4096], BF16, tag="chunk", name=f"t_{b}_{ci}")
                for ci in range(4)
            ]
            # ---------- proj of sample b-2 (all inputs ready) ----------
            if b - 2 in states:
                _, negc, W4 = vstate[b - 2]
                emit_proj(b - 2, states.pop(b - 2), W4, negc)
            # ---------- casts: ALL on DVE before extr/eig (ACT's queue
            # cycles through the sv-copies; DVE's does not) ----------
            for h in range(8):
                ci, h2 = divmod(h, 2)
                nc.vector.tensor_copy(
                    ctiles[ci][:, h2 * 2048:(h2 + 1) * 2048], ftiles[h][:])
            if b - 1 in states:
                psS = emit_extraction(vstate.pop(("psg", b - 1)))
                vstate[b - 1] = emit_eig(b - 1, psS)
            # ---------- gram (bf16) ----------
            psg = psum.tile([128, 128], F32, tag="psg")
            for ci in range(4):
                for j in range(32):
                    lhs = ctiles[ci][:, j * 128:(j + 1) * 128]
                    nc.tensor.matmul(psg[:], lhs, lhs,
                                     start=(ci == 0 and j == 0),
                                     stop=(ci == 3 and j == 31))
            states[b] = ctiles
            vstate[("psg", b)] = psg
        # ---------- epilogue ----------
        b = BPC
        _, negc, W4 = vstate[b - 2]
        emit_proj(b - 2, states.pop(b - 2), W4, negc)
        psS = emit_extraction(vstate.pop(("psg", b - 1)))
        vstate[b - 1] = emit_eig(b - 1, psS)
        _, negc, W4 = vstate[b - 1]
        emit_proj(b - 1, states.pop(b - 1), W4, negc)
    nc.compile()
    return nc


_CACHE = {}


def _get(name, builder):
    if name not in _CACHE:
        _CACHE[name] = builder()
    return _CACHE[name]


def make_aux(mean):
    """mean: [BPC, 4] float -> aux array [128, AUXW]."""
    auxv = np.zeros((128, AUXW), np.float32)
    for b in range(BPC):
        base = 24 * b
        auxv[:, base:base + 4] = -mean[b].astype(np.float32)
        auxv[:, base + 4:base + 20] = np.outer(
            mean[b], mean[b]).astype(np.float32).reshape(16)
    auxv[:, 24 * BPC:24 * BPC + 4] = np.asarray(EVEC, np.float32)
    return auxv


def kernel(inputs: np.ndarray) -> np.ndarray:
    xx = np.ascontiguousarray(np.asarray(inputs, dtype=np.float32))
    assert xx.shape == (B, H, W, C), xx.shape
    xf = xx.reshape(N_CORES, BPC * SAMPLE)
    cores = list(range(N_CORES))
    mean = xx.reshape(B, NROWS, 4).mean(axis=1, dtype=np.float64)  # [B, 4]

    nc = _get("fused", _build_fused)
    in_maps = [
        {"x": xf[c], "aux": make_aux(mean[c * BPC:(c + 1) * BPC])} for c in cores
    ]
    r = run_bass_kernel_spmd(nc, in_maps, cores)
    stats = np.stack([r.results[c]["stats"] for c in cores]).reshape(B, 20)
    yv = np.stack(
        [np.asarray(r.results[c]["y"]).astype(np.float32) for c in cores]
    ).reshape(B, OUT_SAMPLE)

    import ml_dtypes
    S = stats[:, 0:16].reshape(B, 4, 4).astype(np.float64)
    # device projects with bf16-rounded v (diag stationaries); mirror that
    v_dev = stats[:, 16:20].astype(ml_dtypes.bfloat16).astype(np.float64)
    cov = (S / NROWS - np.einsum("bi,bj->bij", mean, mean)).astype(np.float32)

    import jax
    import jax.numpy as jnp
    with jax.default_device(jax.devices("cpu")[0]):
        _, vecs = jnp.linalg.eigh(jnp.asarray(cov))
    v_ref = np.asarray(vecs)[:, :, -1].astype(np.float64)

    dot = (v_ref * v_dev).sum(1)
    scale = np.sign(dot) / np.linalg.norm(v_dev, axis=1)
    yv = (yv * scale[:, None]).astype(np.float32)
    return yv.reshape(B, H // 2, W // 2, C)


# revision 57
# speedup vs baseline: 1.1366x; 1.1366x over previous
"""Fused single-launch BPCA pooling: bf16 gram + on-device top-eigenvector
(shifted power iteration by repeated squaring) + bf16 projection.

v6: lag-2 software pipeline.  body(b) = loads(b) | proj(b-2) |
extraction+eig(b-1) | casts(b) | gram(b).  All proj/plane inputs are
ready at body start (v_rep computed a full period earlier), the gram is
bf16 (fp32 sustained matmul streams throttle ~2x), and the eigensolve
is a DVE-only chain: L-inf shift (no sqrt -> no ACT round trip), one
1/||C||^2 renorm, host-supplied mu-outer-product and -mu, accum_out
fused reductions, stats written by direct DMA.

Host fixes sign/scale of the output using the returned S and
unnormalized v_dev (jax-cpu eigh for the reference LAPACK sign
convention).
"""

import numpy as np
from contextlib import ExitStack

import concourse.bass as bass
import concourse.tile as tile
from concourse import bacc, mybir
from concourse.bass_utils import run_bass_kernel_spmd

B, H, W, C = 32, 64, 64, 512
N_CORES = 8
BPC = B // N_CORES
SAMPLE = H * W * C
NROWS = SAMPLE // 4
OUT_SAMPLE = SAMPLE // 4
F32 = mybir.dt.float32
BF16 = mybir.dt.bfloat16
I32 = mybir.dt.int32
ALU = mybir.AluOpType
AF = mybir.ActivationFunctionType
AXL = mybir.AxisListType

NSQ = 7                       # squarings; top-eig contamination ~ratio^-128
EVEC = [0.9129, -0.6011, 0.3683, 1.0577]   # fixed generic seed vector
AUXW = 24 * BPC + 8


def _in_dram_ap(x, b, half, q, h2):
    off = b * SAMPLE + half * 32768 + q * 4096 + h2 * 2048
    return bass.AP(x, off, [[65536, 32], [8192, 4], [1, 2048]])


def _v(ap, axes, extra_off=0):
    """Free-dim view of a [128, F] (or [P, F]) tile AP with custom free axes."""
    return bass.AP(ap.tensor, ap.offset + extra_off, [list(ap.ap[0])] + axes)


def _build_fused():
    nc = bacc.Bacc("TRN2", target_bir_lowering=False, debug=False)
    x = nc.dram_tensor("x", [BPC * SAMPLE], F32, kind="ExternalInput")
    aux = nc.dram_tensor("aux", [128, AUXW], F32, kind="ExternalInput")
    y = nc.dram_tensor("y", [BPC * OUT_SAMPLE], BF16, kind="ExternalOutput")
    st = nc.dram_tensor("stats", [BPC, 20], F32, kind="ExternalOutput")

    with tile.TileContext(nc) as tc, ExitStack() as ctx:
        const = ctx.enter_context(tc.tile_pool(name="const", bufs=1))
        stage = ctx.enter_context(tc.tile_pool(name="stage", bufs=10))
        chunks = ctx.enter_context(tc.tile_pool(name="chunks", bufs=12))
        psum = ctx.enter_context(tc.tile_pool(name="psum", bufs=2, space="PSUM"))
        psx = ctx.enter_context(tc.tile_pool(name="psx", bufs=1, space="PSUM"))
        red = ctx.enter_context(tc.tile_pool(name="red", bufs=2))
        eig = ctx.enter_context(tc.tile_pool(name="eig", bufs=2))
        stag = ctx.enter_context(tc.tile_pool(name="stag", bufs=1))
        wpool = ctx.enter_context(tc.tile_pool(name="wpool", bufs=2))
        psv = ctx.enter_context(tc.tile_pool(name="psv", bufs=4, space="PSUM"))

        # ---------- constants ----------
        auxp = const.tile([128, AUXW], F32)
        nc.sync.dma_start(auxp[:], bass.AP(aux, 0, [[AUXW, 128], [1, AUXW]]))
        pidx_i = const.tile([128, 1], I32)
        nc.gpsimd.iota(pidx_i[:], [[0, 1]], base=0, channel_multiplier=1)
        pmod_i = const.tile([128, 1], I32)
        nc.vector.tensor_scalar(pmod_i[:], pidx_i[:], 3, None, ALU.bitwise_and)
        pgrp_i = const.tile([128, 1], I32)
        nc.vector.tensor_scalar(pgrp_i[:], pidx_i[:], -4, None, ALU.bitwise_and)
        E4_i = const.tile([128, 4], I32)
        for k in range(4):
            nc.vector.tensor_scalar(E4_i[:, k:k + 1], pmod_i[:], k, None, ALU.is_equal)
        E4 = const.tile([128, 4], F32)
        nc.vector.tensor_copy(E4[:], E4_i[:])
        # mask[p, n] = (n//4 == p//4), [128, 128]
        cidx_i = const.tile([128, 128], I32)
        nc.gpsimd.iota(cidx_i[:], [[1, 128]], base=0, channel_multiplier=0)
        cgrp_i = const.tile([128, 128], I32)
        nc.vector.tensor_scalar(cgrp_i[:], cidx_i[:], -4, None, ALU.bitwise_and)
        cgrp = const.tile([128, 128], F32)
        nc.vector.tensor_copy(cgrp[:], cgrp_i[:])
        pgrp = const.tile([128, 1], F32)
        nc.vector.tensor_copy(pgrp[:], pgrp_i[:])
        mask = const.tile([128, 128], F32)
        nc.vector.tensor_scalar(mask[:], cgrp[:], pgrp[:], 0.0, ALU.subtract, ALU.is_equal)
        # FM[p, u] = (u//4 == p), u in [0,16)  (only partitions 0..3 used)
        uidx_i = const.tile([128, 16], I32)
        nc.gpsimd.iota(uidx_i[:], [[1, 16]], base=0, channel_multiplier=0)
        ugrp_i = const.tile([128, 16], I32)
        nc.vector.tensor_scalar(ugrp_i[:], uidx_i[:], -4, None, ALU.bitwise_and)
        ugrp = const.tile([128, 16], F32)   # 4*(u//4)
        nc.vector.tensor_copy(ugrp[:], ugrp_i[:])
        pidx4 = const.tile([128, 1], F32)   # p*4
        nc.vector.tensor_copy(pidx4[:], pidx_i[:])
        nc.vector.tensor_scalar(pidx4[:], pidx4[:], 4.0, None, ALU.mult)
        FM = const.tile([128, 16], F32)
        nc.vector.tensor_scalar(FM[:], ugrp[:], pidx4[:], 0.0, ALU.subtract, ALU.is_equal)
        # dm16[p, u] = (u//4 == u%4): flat identity; dm16qn = -identity/4
        umod_i = const.tile([128, 16], I32)
        nc.vector.tensor_scalar(umod_i[:], uidx_i[:], 3, None, ALU.bitwise_and)
        ud_i = const.tile([128, 16], I32)
        nc.vector.tensor_scalar(ud_i[:], ugrp_i[:], 2, None, ALU.arith_shift_right)
        umod = const.tile([128, 16], F32)
        nc.vector.tensor_copy(umod[:], umod_i[:])
        ud = const.tile([128, 16], F32)
        nc.vector.tensor_copy(ud[:], ud_i[:])
        dm16 = const.tile([128, 16], F32)
        nc.vector.tensor_tensor(dm16[:], ud[:], umod[:], ALU.is_equal)
        dm16qn = const.tile([128, 16], F32)
        nc.vector.tensor_scalar(dm16qn[:], dm16[:], -0.25, None, ALU.mult)
        ones4x128 = const.tile([4, 128], F32)
        nc.vector.memset(ones4x128[:], 1.0)
        # full [128,128] identity in bf16 (stationary base for diag(v_k))
        pidf = const.tile([128, 1], F32)
        nc.vector.tensor_copy(pidf[:], pidx_i[:])
        cidf = const.tile([128, 128], F32)
        nc.vector.tensor_copy(cidf[:], cidx_i[:])
        idf = const.tile([128, 128], F32)
        nc.vector.tensor_scalar(idf[:], cidf[:], pidf[:], 0.0, ALU.subtract, ALU.is_equal)
        idb = const.tile([128, 128], BF16)
        nc.vector.tensor_copy(idb[:], idf[:])

        def emit_extraction(psg):
            m = red.tile([128, 128], F32, tag="m")
            nc.vector.tensor_mul(m[:], psg[:], mask[:])
            psE = psx.tile([4, 128], F32, tag="psE")
            nc.tensor.matmul(psE[:], E4[:], m[:], start=True, stop=True)
            # S44[k, l] = sum_c psE[k, 4c+l]: strided-innermost reduce from PSUM
            S44 = red.tile([4, 4], F32, tag="S44")
            nc.vector.tensor_reduce(S44[:], _v(psE[:], [[1, 4], [4, 32]]),
                                    AXL.X, ALU.add)
            # spread S [4,4] -> [4,16] rows: Fm16[p,(j,l)] = S[p,l] * (p==j)
            Fm16 = red.tile([4, 16], F32, tag="Fm16")
            s_b = _v(S44[:], [[0, 4], [1, 4]])       # [4, j(bcast), l]
            nc.vector.tensor_tensor(Fm16[:].rearrange("p (j l) -> p j l", j=4),
                                    s_b, FM[0:4, :].rearrange("p (j l) -> p j l", j=4),
                                    ALU.mult)
            psS = psx.tile([128, 16], F32, tag="psS")
            nc.tensor.matmul(psS[:], ones4x128[:], Fm16[:], start=True, stop=True)
            return psS

        def emit_eig(b, psS):
            base = 24 * b
            negmu = auxp[:, base:base + 4]
            mmf = auxp[:, base + 4:base + 20]
            evec = auxp[:, 24 * BPC:24 * BPC + 4]
            covf = eig.tile([128, 16], F32, tag="covf")
            nc.vector.scalar_tensor_tensor(covf[:], psS[:], 1.0 / NROWS, mmf,
                                           ALU.mult, ALU.subtract)
            # -tr/4 via accum over covf * (-I/4)
            dgj = eig.tile([128, 16], F32, tag="dgj")
            trqn = eig.tile([128, 1], F32, tag="trqn")
            nc.vector.scalar_tensor_tensor(dgj[:], covf[:], 1.0, dm16qn[:],
                                           ALU.mult, ALU.mult, accum_out=trqn[:])
            B0 = eig.tile([128, 16], F32, tag="B0")
            nc.vector.scalar_tensor_tensor(B0[:], dm16[:], trqn[:], covf[:],
                                           ALU.mult, ALU.add)
            # shift s = 2*max|B0| (>= spectral bound margin for traceless 4x4)
            mx = eig.tile([128, 1], F32, tag="mx")
            nc.vector.tensor_reduce(mx[:], B0[:], AXL.X, ALU.max)
            mn = eig.tile([128, 1], F32, tag="mn")
            nc.vector.tensor_reduce(mn[:], B0[:], AXL.X, ALU.min)
            nc.vector.tensor_scalar(mx[:], mx[:], 2.0, None, ALU.mult)
            nc.vector.tensor_scalar(mn[:], mn[:], -2.0, None, ALU.mult)
            sft = eig.tile([128, 1], F32, tag="sft")
            nc.vector.tensor_tensor(sft[:], mx[:], mn[:], ALU.max)
            Bc = eig.tile([128, 16], F32, tag="Bc")
            nc.vector.scalar_tensor_tensor(Bc[:], dm16[:], sft[:], B0[:],
                                           ALU.mult, ALU.add)  # B0 + s I
            srec = eig.tile([128, 1], F32, tag="srec")
            nc.vector.reciprocal(srec[:], sft[:])
            nc.vector.tensor_scalar(Bc[:], Bc[:], srec[:], 0.5, ALU.mult, ALU.mult)
            # squarings, ping-pong; single renorm by 1/||C||_F^2 at it==2
            prod = eig.tile([128, 64], F32, tag="prod")
            Cc = eig.tile([128, 16], F32, tag="Cc")
            sqt = eig.tile([128, 16], F32, tag="sqt")
            cur, nxt = Bc, Cc
            for it in range(NSQ):
                b_ik = _v(cur[:], [[4, 4], [0, 4], [1, 4]])  # [p,i,j,k]=B[4i+k]
                b_kj = _v(cur[:], [[0, 4], [1, 4], [4, 4]])  # [p,i,j,k]=B[4k+j]
                nc.vector.tensor_tensor(
                    prod[:].rearrange("p (i j k) -> p i j k", i=4, j=4),
                    b_ik, b_kj, ALU.mult)
                nc.vector.tensor_reduce(
                    nxt[:].rearrange("p (i j) -> p i j", i=4),
                    prod[:].rearrange("p (i j k) -> p i j k", i=4, j=4),
                    AXL.X, ALU.add)
                if it == 2:
                    nrm2 = eig.tile([128, 1], F32, tag="nrm2")
                    nc.vector.scalar_tensor_tensor(
                        sqt[:], nxt[:], 1.0, nxt[:], ALU.mult, ALU.mult,
                        accum_out=nrm2[:])
                    nc.vector.reciprocal(nrm2[:], nrm2[:])
                    nc.vector.tensor_scalar(nxt[:], nxt[:], nrm2[:], None, ALU.mult)
                cur, nxt = nxt, cur
            # v = B @ e  (replicated): v_rep[p, i] = sum_j B[4i+j] e[j]
            vprod = eig.tile([128, 16], F32, tag="vprod")
            nc.vector.tensor_tensor(
                vprod[:].rearrange("p (i j) -> p i j", i=4),
                _v(cur[:], [[4, 4], [1, 4]]), _v(evec, [[0, 4], [1, 4]]), ALU.mult)
            v_rep = eig.tile([128, 4], F32, tag="v_rep")
            nc.vector.tensor_reduce(
                v_rep[:].rearrange("p (i u) -> p i u", i=4),
                vprod[:].rearrange("p (i j) -> p i j", i=4), AXL.X, ALU.add)
            # negc = v . (-mu) via accum
            mvp = eig.tile([128, 4], F32, tag="mvp")
            negc = eig.tile([128, 1], F32, tag="negc")
            nc.vector.scalar_tensor_tensor(mvp[:], v_rep[:], 1.0, negmu,
                                           ALU.mult, ALU.mult, accum_out=negc[:])
            # stats out: [1, 20] = Sflat | v_dev (DMA issued from gpsimd so
            # the sync queue's load triggers never wait on the eig chain)
            stt = eig.tile([1, 20], F32, tag="stt")
            nc.vector.tensor_copy(stt[:, 0:16], psS[0:1, :])
            nc.vector.tensor_copy(stt[:, 16:20], v_rep[0:1, :])
            nc.gpsimd.dma_start(bass.AP(st, b * 20, [[20, 1], [1, 20]]), stt[:])
            # stationary diag(v_k) bf16 matrices for the PE projection
            W4 = []
            for k in range(4):
                Wk = wpool.tile([128, 128], BF16, tag=f"W{k}", name=f"W_{b}_{k}")
                nc.vector.tensor_scalar(Wk[:], idb[:], v_rep[:, k:k + 1], None,
                                        ALU.mult)
                W4.append(Wk)
            return v_rep, negc, W4

        def emit_proj(pb, pctiles, W4, pnegc):
            # PE projection: per PSUM bank, 4 accumulating matmuls with
            # stationary diag(v_k) and moving = strided channel-k view;
            # one ACT op applies -c and writes bf16 to stg; gpsimd DMAs out.
            stg = stag.tile([128, 4096], BF16, tag="stg", name=f"stg_{pb}")
            for ci, t in enumerate(pctiles):
                half, q = divmod(ci, 2)
                for hf in range(2):
                    ps = psv.tile([128, 512], F32, tag="sv",
                                  name=f"sv_{pb}_{ci}_{hf}")
                    for k in range(4):
                        rhs = bass.AP(t[:].tensor,
                                      t[:].offset + hf * 2048 + k,
                                      [list(t[:].ap[0])] + [[4, 512]])
                        nc.tensor.matmul(ps[:], W4[k][:], rhs,
                                         start=(k == 0), stop=(k == 3))
                    nc.scalar.activation(
                        stg[:, ci * 1024 + hf * 512:ci * 1024 + (hf + 1) * 512],
                        ps[:], AF.Identity, bias=pnegc[:], scale=1.0)
                nc.gpsimd.dma_start(
                    bass.AP(y, pb * OUT_SAMPLE + q * 2048 + half * 256,
                            [[4096, 128], [512, 4], [1, 256]]),
                    _v(stg[:], [[256, 4], [1, 256]], extra_off=ci * 1024))

        # Lag-2 pipeline.
        states = {}
        vstate = {}
        for b in range(BPC):
            # ---------- loads: 8 half-chunk staging tiles; triggers split
            # across two queues to halve head-of-line blocking ----------
            ftiles = []
            for h in range(8):
                ci, h2 = divmod(h, 2)
                half, q = divmod(ci, 2)
                tf = stage.tile([128, 2048], F32, tag="stagef", name=f"tf_{b}_{h}")
                nc.sync.dma_start(tf[:], _in_dram_ap(x, b, half, q, h2))
                ftiles.append(tf)
            ctiles = [
                chunks.tile([128, 4096], BF16, tag="chunk", name=f"t_{b}_{ci}")
                for ci in range(4)
            ]
            # ---------- proj of sample b-2 (all inputs ready) ----------
            if b - 2 in states:
                _, negc, W4 = vstate[b - 2]
                emit_proj(b - 2, states.pop(b - 2), W4, negc)
            # ---------- casts: ALL on DVE before extr/eig (ACT's queue
            # cycles through the sv-copies; DVE's does not) ----------
            for h in range(8):
                ci, h2 = divmod(h, 2)
                nc.vector.tensor_copy(
                    ctiles[ci][:, h2 * 2048:(h2 + 1) * 2048], ftiles[h][:])
            if b - 1 in states:
                psS = emit_extraction(vstate.pop(("psg", b - 1)))
                vstate[b - 1] = emit_eig(b - 1, psS)
            # ---------- gram (bf16) ----------
            psg = psum.tile([128, 128], F32, tag="psg")
            for ci in range(4):
                for j in range(32):
                    lhs = ctiles[ci][:, j * 128:(j + 1) * 128]
                    nc.tensor.matmul(psg[:], lhs, lhs,
                                     start=(ci == 0 and j == 0),
                                     stop=(ci == 3 and j == 31))
            states[b] = ctiles
            vstate[("psg", b)] = psg
        # ---------- epilogue ----------
        b = BPC
        _, negc, W4 = vstate[b - 2]
        emit_proj(b - 2, states.pop(b - 2), W4, negc)
        psS = emit_extraction(vstate.pop(("psg", b - 1)))
        vstate[b - 1] = emit_eig(b - 1, psS)
        _, negc, W4 = vstate[b - 1]
        emit_proj(b - 1, states.pop(b - 1), W4, negc)
    nc.compile()
    return nc


_CACHE = {}


def _get(name, builder):
    if name not in _CACHE:
        _CACHE[name] = builder()
    return _CACHE[name]


def make_aux(mean):
    """mean: [BPC, 4] float -> aux array [128, AUXW]."""
    auxv = np.zeros((128, AUXW), np.float32)
    for b in range(BPC):
        base = 24 * b
        auxv[:, base:base + 4] = -mean[b].astype(np.float32)
        auxv[:, base + 4:base + 20] = np.outer(
            mean[b], mean[b]).astype(np.float32).reshape(16)
    auxv[:, 24 * BPC:24 * BPC + 4] = np.asarray(EVEC, np.float32)
    return auxv


def kernel(inputs: np.ndarray) -> np.ndarray:
    xx = np.ascontiguousarray(np.asarray(inputs, dtype=np.float32))
    assert xx.shape == (B, H, W, C), xx.shape
    xf = xx.reshape(N_CORES, BPC * SAMPLE)
    cores = list(range(N_CORES))
    mean = xx.reshape(B, NROWS, 4).mean(axis=1, dtype=np.float64)  # [B, 4]

    nc = _get("fused", _build_fused)
    in_maps = [
        {"x": xf[c], "aux": make_aux(mean[c * BPC:(c + 1) * BPC])} for c in cores
    ]
    r = run_bass_kernel_spmd(nc, in_maps, cores)
    stats = np.stack([r.results[c]["stats"] for c in cores]).reshape(B, 20)
    yv = np.stack(
        [np.asarray(r.results[c]["y"]).astype(np.float32) for c in cores]
    ).reshape(B, OUT_SAMPLE)

    import ml_dtypes
    S = stats[:, 0:16].reshape(B, 4, 4).astype(np.float64)
    # device projects with bf16-rounded v (diag stationaries); mirror that
    v_dev = stats[:, 16:20].astype(ml_dtypes.bfloat16).astype(np.float64)
    cov = (S / NROWS - np.einsum("bi,bj->bij", mean, mean)).astype(np.float32)

    import jax
    import jax.numpy as jnp
    with jax.default_device(jax.devices("cpu")[0]):
        _, vecs = jnp.linalg.eigh(jnp.asarray(cov))
    v_ref = np.asarray(vecs)[:, :, -1].astype(np.float64)

    dot = (v_ref * v_dev).sum(1)
    scale = np.sign(dot) / np.linalg.norm(v_dev, axis=1)
    yv = (yv * scale[:, None]).astype(np.float32)
    return yv.reshape(B, H // 2, W // 2, C)


# revision 59
# speedup vs baseline: 1.1440x; 1.0065x over previous
"""Fused single-launch BPCA pooling: bf16 gram + on-device top-eigenvector
(shifted power iteration by repeated squaring) + bf16 projection.

v6: lag-2 software pipeline.  body(b) = loads(b) | proj(b-2) |
extraction+eig(b-1) | casts(b) | gram(b).  All proj/plane inputs are
ready at body start (v_rep computed a full period earlier), the gram is
bf16 (fp32 sustained matmul streams throttle ~2x), and the eigensolve
is a DVE-only chain: L-inf shift (no sqrt -> no ACT round trip), one
1/||C||^2 renorm, host-supplied mu-outer-product and -mu, accum_out
fused reductions, stats written by direct DMA.

Host fixes sign/scale of the output using the returned S and
unnormalized v_dev (jax-cpu eigh for the reference LAPACK sign
convention).
"""

import numpy as np
from contextlib import ExitStack

import concourse.bass as bass
import concourse.tile as tile
from concourse import bacc, mybir
from concourse.bass_utils import run_bass_kernel_spmd

B, H, W, C = 32, 64, 64, 512
N_CORES = 8
BPC = B // N_CORES
SAMPLE = H * W * C
NROWS = SAMPLE // 4
OUT_SAMPLE = SAMPLE // 4
F32 = mybir.dt.float32
BF16 = mybir.dt.bfloat16
I32 = mybir.dt.int32
ALU = mybir.AluOpType
AF = mybir.ActivationFunctionType
AXL = mybir.AxisListType

NSQ = 7                       # squarings; top-eig contamination ~ratio^-128
EVEC = [0.9129, -0.6011, 0.3683, 1.0577]   # fixed generic seed vector
AUXW = 24 * BPC + 8


def _in_dram_ap(x, b, half, q, h2):
    off = b * SAMPLE + half * 32768 + q * 4096 + h2 * 2048
    return bass.AP(x, off, [[65536, 32], [8192, 4], [1, 2048]])


def _v(ap, axes, extra_off=0):
    """Free-dim view of a [128, F] (or [P, F]) tile AP with custom free axes."""
    return bass.AP(ap.tensor, ap.offset + extra_off, [list(ap.ap[0])] + axes)


def _build_fused():
    nc = bacc.Bacc("TRN2", target_bir_lowering=False, debug=False)
    x = nc.dram_tensor("x", [BPC * SAMPLE], F32, kind="ExternalInput")
    aux = nc.dram_tensor("aux", [128, AUXW], F32, kind="ExternalInput")
    y = nc.dram_tensor("y", [BPC * OUT_SAMPLE], BF16, kind="ExternalOutput")
    st = nc.dram_tensor("stats", [BPC, 20], F32, kind="ExternalOutput")

    with tile.TileContext(nc) as tc, ExitStack() as ctx:
        const = ctx.enter_context(tc.tile_pool(name="const", bufs=1))
        stage = ctx.enter_context(tc.tile_pool(name="stage", bufs=10))
        chunks = ctx.enter_context(tc.tile_pool(name="chunks", bufs=12))
        psum = ctx.enter_context(tc.tile_pool(name="psum", bufs=2, space="PSUM"))
        psx = ctx.enter_context(tc.tile_pool(name="psx", bufs=1, space="PSUM"))
        red = ctx.enter_context(tc.tile_pool(name="red", bufs=2))
        eig = ctx.enter_context(tc.tile_pool(name="eig", bufs=2))
        stag = ctx.enter_context(tc.tile_pool(name="stag", bufs=1))
        wpool = ctx.enter_context(tc.tile_pool(name="wpool", bufs=2))
        psv = ctx.enter_context(tc.tile_pool(name="psv", bufs=4, space="PSUM"))

        # ---------- constants ----------
        auxp = const.tile([128, AUXW], F32)
        nc.sync.dma_start(auxp[:], bass.AP(aux, 0, [[AUXW, 128], [1, AUXW]]))
        pidx_i = const.tile([128, 1], I32)
        nc.gpsimd.iota(pidx_i[:], [[0, 1]], base=0, channel_multiplier=1)
        pmod_i = const.tile([128, 1], I32)
        nc.vector.tensor_scalar(pmod_i[:], pidx_i[:], 3, None, ALU.bitwise_and)
        pgrp_i = const.tile([128, 1], I32)
        nc.vector.tensor_scalar(pgrp_i[:], pidx_i[:], -4, None, ALU.bitwise_and)
        E4_i = const.tile([128, 4], I32)
        for k in range(4):
            nc.vector.tensor_scalar(E4_i[:, k:k + 1], pmod_i[:], k, None, ALU.is_equal)
        E4 = const.tile([128, 4], F32)
        nc.vector.tensor_copy(E4[:], E4_i[:])
        # mask[p, n] = (n//4 == p//4), [128, 128]
        cidx_i = const.tile([128, 128], I32)
        nc.gpsimd.iota(cidx_i[:], [[1, 128]], base=0, channel_multiplier=0)
        cgrp_i = const.tile([128, 128], I32)
        nc.vector.tensor_scalar(cgrp_i[:], cidx_i[:], -4, None, ALU.bitwise_and)
        cgrp = const.tile([128, 128], F32)
        nc.vector.tensor_copy(cgrp[:], cgrp_i[:])
        pgrp = const.tile([128, 1], F32)
        nc.vector.tensor_copy(pgrp[:], pgrp_i[:])
        mask = const.tile([128, 128], F32)
        nc.vector.tensor_scalar(mask[:], cgrp[:], pgrp[:], 0.0, ALU.subtract, ALU.is_equal)
        # FM[p, u] = (u//4 == p), u in [0,16)  (only partitions 0..3 used)
        uidx_i = const.tile([128, 16], I32)
        nc.gpsimd.iota(uidx_i[:], [[1, 16]], base=0, channel_multiplier=0)
        ugrp_i = const.tile([128, 16], I32)
        nc.vector.tensor_scalar(ugrp_i[:], uidx_i[:], -4, None, ALU.bitwise_and)
        ugrp = const.tile([128, 16], F32)   # 4*(u//4)
        nc.vector.tensor_copy(ugrp[:], ugrp_i[:])
        pidx4 = const.tile([128, 1], F32)   # p*4
        nc.vector.tensor_copy(pidx4[:], pidx_i[:])
        nc.vector.tensor_scalar(pidx4[:], pidx4[:], 4.0, None, ALU.mult)
        FM = const.tile([128, 16], F32)
        nc.vector.tensor_scalar(FM[:], ugrp[:], pidx4[:], 0.0, ALU.subtract, ALU.is_equal)
        # dm16[p, u] = (u//4 == u%4): flat identity; dm16qn = -identity/4
        umod_i = const.tile([128, 16], I32)
        nc.vector.tensor_scalar(umod_i[:], uidx_i[:], 3, None, ALU.bitwise_and)
        ud_i = const.tile([128, 16], I32)
        nc.vector.tensor_scalar(ud_i[:], ugrp_i[:], 2, None, ALU.arith_shift_right)
        umod = const.tile([128, 16], F32)
        nc.vector.tensor_copy(umod[:], umod_i[:])
        ud = const.tile([128, 16], F32)
        nc.vector.tensor_copy(ud[:], ud_i[:])
        dm16 = const.tile([128, 16], F32)
        nc.vector.tensor_tensor(dm16[:], ud[:], umod[:], ALU.is_equal)
        dm16qn = const.tile([128, 16], F32)
        nc.vector.tensor_scalar(dm16qn[:], dm16[:], -0.25, None, ALU.mult)
        ones4x128 = const.tile([4, 128], F32)
        nc.vector.memset(ones4x128[:], 1.0)
        # full [128,128] identity in bf16 (stationary base for diag(v_k))
        pidf = const.tile([128, 1], F32)
        nc.vector.tensor_copy(pidf[:], pidx_i[:])
        cidf = const.tile([128, 128], F32)
        nc.vector.tensor_copy(cidf[:], cidx_i[:])
        idf = const.tile([128, 128], F32)
        nc.vector.tensor_scalar(idf[:], cidf[:], pidf[:], 0.0, ALU.subtract, ALU.is_equal)
        idb = const.tile([128, 128], BF16)
        nc.vector.tensor_copy(idb[:], idf[:])

        def emit_extraction(psg):
            m = red.tile([128, 128], F32, tag="m")
            nc.vector.tensor_mul(m[:], psg[:], mask[:])
            psE = psx.tile([4, 128], F32, tag="psE")
            nc.tensor.matmul(psE[:], E4[:], m[:], start=True, stop=True)
            # S44[k, l] = sum_c psE[k, 4c+l]: strided-innermost reduce from PSUM
            S44 = red.tile([4, 4], F32, tag="S44")
            nc.vector.tensor_reduce(S44[:], _v(psE[:], [[1, 4], [4, 32]]),
                                    AXL.X, ALU.add)
            # spread S [4,4] -> [4,16] rows: Fm16[p,(j,l)] = S[p,l] * (p==j)
            Fm16 = red.tile([4, 16], F32, tag="Fm16")
            s_b = _v(S44[:], [[0, 4], [1, 4]])       # [4, j(bcast), l]
            nc.vector.tensor_tensor(Fm16[:].rearrange("p (j l) -> p j l", j=4),
                                    s_b, FM[0:4, :].rearrange("p (j l) -> p j l", j=4),
                                    ALU.mult)
            psS = psx.tile([128, 16], F32, tag="psS")
            nc.tensor.matmul(psS[:], ones4x128[:], Fm16[:], start=True, stop=True)
            return psS

        def emit_eig(b, psS):
            base = 24 * b
            negmu = auxp[:, base:base + 4]
            mmf = auxp[:, base + 4:base + 20]
            evec = auxp[:, 24 * BPC:24 * BPC + 4]
            covf = eig.tile([128, 16], F32, tag="covf")
            nc.vector.scalar_tensor_tensor(covf[:], psS[:], 1.0 / NROWS, mmf,
                                           ALU.mult, ALU.subtract)
            # -tr/4 via accum over covf * (-I/4)
            dgj = eig.tile([128, 16], F32, tag="dgj")
            trqn = eig.tile([128, 1], F32, tag="trqn")
            nc.vector.scalar_tensor_tensor(dgj[:], covf[:], 1.0, dm16qn[:],
                                           ALU.mult, ALU.mult, accum_out=trqn[:])
            B0 = eig.tile([128, 16], F32, tag="B0")
            nc.vector.scalar_tensor_tensor(B0[:], dm16[:], trqn[:], covf[:],
                                           ALU.mult, ALU.add)
            # shift s = 2*max|B0| (>= spectral bound margin for traceless 4x4)
            mx = eig.tile([128, 1], F32, tag="mx")
            nc.vector.tensor_reduce(mx[:], B0[:], AXL.X, ALU.max)
            mn = eig.tile([128, 1], F32, tag="mn")
            nc.vector.tensor_reduce(mn[:], B0[:], AXL.X, ALU.min)
            nc.vector.tensor_scalar(mx[:], mx[:], 2.0, None, ALU.mult)
            nc.vector.tensor_scalar(mn[:], mn[:], -2.0, None, ALU.mult)
            sft = eig.tile([128, 1], F32, tag="sft")
            nc.vector.tensor_tensor(sft[:], mx[:], mn[:], ALU.max)
            Bc = eig.tile([128, 16], F32, tag="Bc")
            nc.vector.scalar_tensor_tensor(Bc[:], dm16[:], sft[:], B0[:],
                                           ALU.mult, ALU.add)  # B0 + s I
            srec = eig.tile([128, 1], F32, tag="srec")
            nc.vector.reciprocal(srec[:], sft[:])
            nc.vector.tensor_scalar(Bc[:], Bc[:], srec[:], 0.5, ALU.mult, ALU.mult)
            # squarings, ping-pong; single renorm by 1/||C||_F^2 at it==2
            prod = eig.tile([128, 64], F32, tag="prod")
            Cc = eig.tile([128, 16], F32, tag="Cc")
            sqt = eig.tile([128, 16], F32, tag="sqt")
            cur, nxt = Bc, Cc
            for it in range(NSQ):
                b_ik = _v(cur[:], [[4, 4], [0, 4], [1, 4]])  # [p,i,j,k]=B[4i+k]
                b_kj = _v(cur[:], [[0, 4], [1, 4], [4, 4]])  # [p,i,j,k]=B[4k+j]
                nc.vector.tensor_tensor(
                    prod[:].rearrange("p (i j k) -> p i j k", i=4, j=4),
                    b_ik, b_kj, ALU.mult)
                nc.vector.tensor_reduce(
                    nxt[:].rearrange("p (i j) -> p i j", i=4),
                    prod[:].rearrange("p (i j k) -> p i j k", i=4, j=4),
                    AXL.X, ALU.add)
                if it == 2:
                    nrm2 = eig.tile([128, 1], F32, tag="nrm2")
                    nc.vector.scalar_tensor_tensor(
                        sqt[:], nxt[:], 1.0, nxt[:], ALU.mult, ALU.mult,
                        accum_out=nrm2[:])
                    nc.vector.reciprocal(nrm2[:], nrm2[:])
                    nc.vector.tensor_scalar(nxt[:], nxt[:], nrm2[:], None, ALU.mult)
                cur, nxt = nxt, cur
            # v = B @ e  (replicated): v_rep[p, i] = sum_j B[4i+j] e[j]
            vprod = eig.tile([128, 16], F32, tag="vprod")
            nc.vector.tensor_tensor(
                vprod[:].rearrange("p (i j) -> p i j", i=4),
                _v(cur[:], [[4, 4], [1, 4]]), _v(evec, [[0, 4], [1, 4]]), ALU.mult)
            v_rep = eig.tile([128, 4], F32, tag="v_rep")
            nc.vector.tensor_reduce(
                v_rep[:].rearrange("p (i u) -> p i u", i=4),
                vprod[:].rearrange("p (i j) -> p i j", i=4), AXL.X, ALU.add)
            # negc = v . (-mu) via accum
            mvp = eig.tile([128, 4], F32, tag="mvp")
            negc = eig.tile([128, 1], F32, tag="negc")
            nc.vector.scalar_tensor_tensor(mvp[:], v_rep[:], 1.0, negmu,
                                           ALU.mult, ALU.mult, accum_out=negc[:])
            # stats out: [1, 20] = Sflat | v_dev (DMA issued from gpsimd so
            # the sync queue's load triggers never wait on the eig chain)
            stt = eig.tile([1, 20], F32, tag="stt")
            nc.vector.tensor_copy(stt[:, 0:16], psS[0:1, :])
            nc.vector.tensor_copy(stt[:, 16:20], v_rep[0:1, :])
            nc.gpsimd.dma_start(bass.AP(st, b * 20, [[20, 1], [1, 20]]), stt[:])
            # stationary diag(v_k) bf16 matrices for the PE projection
            W4 = []
            for k in range(4):
                Wk = wpool.tile([128, 128], BF16, tag=f"W{k}", name=f"W_{b}_{k}")
                nc.vector.tensor_scalar(Wk[:], idb[:], v_rep[:, k:k + 1], None,
                                        ALU.mult)
                W4.append(Wk)
            return v_rep, negc, W4

        def emit_proj_chunk(pb, ci, t, W4, pnegc, stg):
            # PE projection of one chunk: per PSUM bank, 4 accumulating
            # matmuls with stationary diag(v_k), moving = strided channel-k
            # view; one ACT op applies -c and writes bf16; gpsimd DMAs out.
            half, q = divmod(ci, 2)
            for hf in range(2):
                ps = psv.tile([128, 512], F32, tag="sv",
                              name=f"sv_{pb}_{ci}_{hf}")
                for k in range(4):
                    rhs = bass.AP(t[:].tensor,
                                  t[:].offset + hf * 2048 + k,
                                  [list(t[:].ap[0])] + [[4, 512]])
                    nc.tensor.matmul(ps[:], W4[k][:], rhs,
                                     start=(k == 0), stop=(k == 3))
                nc.scalar.activation(
                    stg[:, ci * 1024 + hf * 512:ci * 1024 + (hf + 1) * 512],
                    ps[:], AF.Identity, bias=pnegc[:], scale=1.0)
            nc.gpsimd.dma_start(
                bass.AP(y, pb * OUT_SAMPLE + q * 2048 + half * 256,
                        [[4096, 128], [512, 4], [1, 256]]),
                _v(stg[:], [[256, 4], [1, 256]], extra_off=ci * 1024))

        # Lag-2 pipeline.
        states = {}
        vstate = {}
        for b in range(BPC):
            # ---------- loads: 8 half-chunk staging tiles; triggers split
            # across two queues to halve head-of-line blocking ----------
            ftiles = []
            for h in range(8):
                ci, h2 = divmod(h, 2)
                half, q = divmod(ci, 2)
                tf = stage.tile([128, 2048], F32, tag="stagef", name=f"tf_{b}_{h}")
                nc.sync.dma_start(tf[:], _in_dram_ap(x, b, half, q, h2))
                ftiles.append(tf)
            ctiles = [
                chunks.tile([128, 4096], BF16, tag="chunk", name=f"t_{b}_{ci}")
                for ci in range(4)
            ]
            prev2 = states.pop(b - 2, None)
            if prev2 is not None:
                _, pnegc, pW4 = vstate[b - 2]
                pstg = stag.tile([128, 4096], BF16, tag="stg", name=f"stg_{b-2}")
            psg = psum.tile([128, 128], F32, tag="psg")

            def cast_h(h):
                ci, h2 = divmod(h, 2)
                nc.vector.tensor_copy(
                    ctiles[ci][:, h2 * 2048:(h2 + 1) * 2048], ftiles[h][:])

            def gram_c(ci):
                for j in range(32):
                    lhs = ctiles[ci][:, j * 128:(j + 1) * 128]
                    nc.tensor.matmul(psg[:], lhs, lhs,
                                     start=(ci == 0 and j == 0),
                                     stop=(ci == 3 and j == 31))

            # Interleaved emission: keep the PE continuously fed — proj
            # chunks of b-2 alternate with extraction matmuls of b-1 and
            # gram chunks of b; DVE casts and eig weave between.
            if prev2 is not None:
                emit_proj_chunk(b - 2, 0, prev2[0], pW4, pnegc, pstg)
            cast_h(0); cast_h(1)
            psS = None
            if b - 1 in states:
                psS = emit_extraction(vstate.pop(("psg", b - 1)))
            if prev2 is not None:
                emit_proj_chunk(b - 2, 1, prev2[1], pW4, pnegc, pstg)
            cast_h(2); cast_h(3)
            gram_c(0)
            if prev2 is not None:
                emit_proj_chunk(b - 2, 2, prev2[2], pW4, pnegc, pstg)
            cast_h(4); cast_h(5)
            gram_c(1)
            if psS is not None:
                vstate[b - 1] = emit_eig(b - 1, psS)
            if prev2 is not None:
                emit_proj_chunk(b - 2, 3, prev2[3], pW4, pnegc, pstg)
            cast_h(6); cast_h(7)
            gram_c(2)
            gram_c(3)
            states[b] = ctiles
            vstate[("psg", b)] = psg
        # ---------- epilogue ----------
        b = BPC
        _, pnegc, pW4 = vstate[b - 2]
        pstg = stag.tile([128, 4096], BF16, tag="stg", name=f"stg_{b-2}")
        prev2 = states.pop(b - 2)
        psS = emit_extraction(vstate.pop(("psg", b - 1)))
        for ci in range(4):
            emit_proj_chunk(b - 2, ci, prev2[ci], pW4, pnegc, pstg)
        vstate[b - 1] = emit_eig(b - 1, psS)
        _, pnegc, pW4 = vstate[b - 1]
        pstg = stag.tile([128, 4096], BF16, tag="stg", name=f"stg_{b-1}")
        prev1 = states.pop(b - 1)
        for ci in range(4):
            emit_proj_chunk(b - 1, ci, prev1[ci], pW4, pnegc, pstg)
    nc.compile()
    return nc


_CACHE = {}


def _get(name, builder):
    if name not in _CACHE:
        _CACHE[name] = builder()
    return _CACHE[name]


def make_aux(mean):
    """mean: [BPC, 4] float -> aux array [128, AUXW]."""
    auxv = np.zeros((128, AUXW), np.float32)
    for b in range(BPC):
        base = 24 * b
        auxv[:, base:base + 4] = -mean[b].astype(np.float32)
        auxv[:, base + 4:base + 20] = np.outer(
            mean[b], mean[b]).astype(np.float32).reshape(16)
    auxv[:, 24 * BPC:24 * BPC + 4] = np.asarray(EVEC, np.float32)
    return auxv


def kernel(inputs: np.ndarray) -> np.ndarray:
    xx = np.ascontiguousarray(np.asarray(inputs, dtype=np.float32))
    assert xx.shape == (B, H, W, C), xx.shape
    xf = xx.reshape(N_CORES, BPC * SAMPLE)
    cores = list(range(N_CORES))
    mean = xx.reshape(B, NROWS, 4).mean(axis=1, dtype=np.float64)  # [B, 4]

    nc = _get("fused", _build_fused)
    in_maps = [
        {"x": xf[c], "aux": make_aux(mean[c * BPC:(c + 1) * BPC])} for c in cores
    ]
    r = run_bass_kernel_spmd(nc, in_maps, cores)
    stats = np.stack([r.results[c]["stats"] for c in cores]).reshape(B, 20)
    yv = np.stack(
        [np.asarray(r.results[c]["y"]).astype(np.float32) for c in cores]
    ).reshape(B, OUT_SAMPLE)

    import ml_dtypes
    S = stats[:, 0:16].reshape(B, 4, 4).astype(np.float64)
    # device projects with bf16-rounded v (diag stationaries); mirror that
    v_dev = stats[:, 16:20].astype(ml_dtypes.bfloat16).astype(np.float64)
    cov = (S / NROWS - np.einsum("bi,bj->bij", mean, mean)).astype(np.float32)

    import jax
    import jax.numpy as jnp
    with jax.default_device(jax.devices("cpu")[0]):
        _, vecs = jnp.linalg.eigh(jnp.asarray(cov))
    v_ref = np.asarray(vecs)[:, :, -1].astype(np.float64)

    dot = (v_ref * v_dev).sum(1)
    scale = np.sign(dot) / np.linalg.norm(v_dev, axis=1)
    yv = (yv * scale[:, None]).astype(np.float32)
    return yv.reshape(B, H // 2, W // 2, C)


# revision 62
# speedup vs baseline: 1.1701x; 1.0228x over previous
"""Fused single-launch BPCA pooling: bf16 gram + on-device top-eigenvector
(shifted power iteration by repeated squaring) + bf16 projection.

v6: lag-2 software pipeline.  body(b) = loads(b) | proj(b-2) |
extraction+eig(b-1) | casts(b) | gram(b).  All proj/plane inputs are
ready at body start (v_rep computed a full period earlier), the gram is
bf16 (fp32 sustained matmul streams throttle ~2x), and the eigensolve
is a DVE-only chain: L-inf shift (no sqrt -> no ACT round trip), one
1/||C||^2 renorm, host-supplied mu-outer-product and -mu, accum_out
fused reductions, stats written by direct DMA.

Host fixes sign/scale of the output using the returned S and
unnormalized v_dev (jax-cpu eigh for the reference LAPACK sign
convention).
"""

import numpy as np
from contextlib import ExitStack

import concourse.bass as bass
import concourse.tile as tile
from concourse import bacc, mybir
from concourse.bass_utils import run_bass_kernel_spmd

B, H, W, C = 32, 64, 64, 512
N_CORES = 8
BPC = B // N_CORES
SAMPLE = H * W * C
NROWS = SAMPLE // 4
OUT_SAMPLE = SAMPLE // 4
F32 = mybir.dt.float32
BF16 = mybir.dt.bfloat16
I32 = mybir.dt.int32
ALU = mybir.AluOpType
AF = mybir.ActivationFunctionType
AXL = mybir.AxisListType

NSQ = 7                       # squarings; top-eig contamination ~ratio^-128
EVEC = [0.9129, -0.6011, 0.3683, 1.0577]   # fixed generic seed vector
AUXW = 24 * BPC + 8


def _in_dram_ap(x, b, half, q, h2):
    off = b * SAMPLE + half * 32768 + q * 4096 + h2 * 2048
    return bass.AP(x, off, [[65536, 32], [8192, 4], [1, 2048]])


def _v(ap, axes, extra_off=0):
    """Free-dim view of a [128, F] (or [P, F]) tile AP with custom free axes."""
    return bass.AP(ap.tensor, ap.offset + extra_off, [list(ap.ap[0])] + axes)


def _build_fused():
    nc = bacc.Bacc("TRN2", target_bir_lowering=False, debug=False)
    x = nc.dram_tensor("x", [BPC * SAMPLE], F32, kind="ExternalInput")
    aux = nc.dram_tensor("aux", [128, AUXW], F32, kind="ExternalInput")
    y = nc.dram_tensor("y", [BPC * OUT_SAMPLE], BF16, kind="ExternalOutput")
    st = nc.dram_tensor("stats", [BPC, 20], F32, kind="ExternalOutput")

    with tile.TileContext(nc) as tc, ExitStack() as ctx:
        const = ctx.enter_context(tc.tile_pool(name="const", bufs=1))
        stage = ctx.enter_context(tc.tile_pool(name="stage", bufs=11))
        chunks = ctx.enter_context(tc.tile_pool(name="chunks", bufs=11))
        planes = ctx.enter_context(tc.tile_pool(name="planes", bufs=2))
        psum = ctx.enter_context(tc.tile_pool(name="psum", bufs=2, space="PSUM"))
        psx = ctx.enter_context(tc.tile_pool(name="psx", bufs=1, space="PSUM"))
        red = ctx.enter_context(tc.tile_pool(name="red", bufs=2))
        eig = ctx.enter_context(tc.tile_pool(name="eig", bufs=2))
        stag = ctx.enter_context(tc.tile_pool(name="stag", bufs=1))
        wpool = ctx.enter_context(tc.tile_pool(name="wpool", bufs=2))
        psv = ctx.enter_context(tc.tile_pool(name="psv", bufs=4, space="PSUM"))

        # ---------- constants ----------
        auxp = const.tile([128, AUXW], F32)
        nc.sync.dma_start(auxp[:], bass.AP(aux, 0, [[AUXW, 128], [1, AUXW]]))
        pidx_i = const.tile([128, 1], I32)
        nc.gpsimd.iota(pidx_i[:], [[0, 1]], base=0, channel_multiplier=1)
        pmod_i = const.tile([128, 1], I32)
        nc.vector.tensor_scalar(pmod_i[:], pidx_i[:], 3, None, ALU.bitwise_and)
        pgrp_i = const.tile([128, 1], I32)
        nc.vector.tensor_scalar(pgrp_i[:], pidx_i[:], -4, None, ALU.bitwise_and)
        E4_i = const.tile([128, 4], I32)
        for k in range(4):
            nc.vector.tensor_scalar(E4_i[:, k:k + 1], pmod_i[:], k, None, ALU.is_equal)
        E4 = const.tile([128, 4], F32)
        nc.vector.tensor_copy(E4[:], E4_i[:])
        # mask[p, n] = (n//4 == p//4), [128, 128]
        cidx_i = const.tile([128, 128], I32)
        nc.gpsimd.iota(cidx_i[:], [[1, 128]], base=0, channel_multiplier=0)
        cgrp_i = const.tile([128, 128], I32)
        nc.vector.tensor_scalar(cgrp_i[:], cidx_i[:], -4, None, ALU.bitwise_and)
        cgrp = const.tile([128, 128], F32)
        nc.vector.tensor_copy(cgrp[:], cgrp_i[:])
        pgrp = const.tile([128, 1], F32)
        nc.vector.tensor_copy(pgrp[:], pgrp_i[:])
        mask = const.tile([128, 128], F32)
        nc.vector.tensor_scalar(mask[:], cgrp[:], pgrp[:], 0.0, ALU.subtract, ALU.is_equal)
        # FM[p, u] = (u//4 == p), u in [0,16)  (only partitions 0..3 used)
        uidx_i = const.tile([128, 16], I32)
        nc.gpsimd.iota(uidx_i[:], [[1, 16]], base=0, channel_multiplier=0)
        ugrp_i = const.tile([128, 16], I32)
        nc.vector.tensor_scalar(ugrp_i[:], uidx_i[:], -4, None, ALU.bitwise_and)
        ugrp = const.tile([128, 16], F32)   # 4*(u//4)
        nc.vector.tensor_copy(ugrp[:], ugrp_i[:])
        pidx4 = const.tile([128, 1], F32)   # p*4
        nc.vector.tensor_copy(pidx4[:], pidx_i[:])
        nc.vector.tensor_scalar(pidx4[:], pidx4[:], 4.0, None, ALU.mult)
        FM = const.tile([128, 16], F32)
        nc.vector.tensor_scalar(FM[:], ugrp[:], pidx4[:], 0.0, ALU.subtract, ALU.is_equal)
        # dm16[p, u] = (u//4 == u%4): flat identity; dm16qn = -identity/4
        umod_i = const.tile([128, 16], I32)
        nc.vector.tensor_scalar(umod_i[:], uidx_i[:], 3, None, ALU.bitwise_and)
        ud_i = const.tile([128, 16], I32)
        nc.vector.tensor_scalar(ud_i[:], ugrp_i[:], 2, None, ALU.arith_shift_right)
        umod = const.tile([128, 16], F32)
        nc.vector.tensor_copy(umod[:], umod_i[:])
        ud = const.tile([128, 16], F32)
        nc.vector.tensor_copy(ud[:], ud_i[:])
        dm16 = const.tile([128, 16], F32)
        nc.vector.tensor_tensor(dm16[:], ud[:], umod[:], ALU.is_equal)
        dm16qn = const.tile([128, 16], F32)
        nc.vector.tensor_scalar(dm16qn[:], dm16[:], -0.25, None, ALU.mult)
        ones4x128 = const.tile([4, 128], F32)
        nc.vector.memset(ones4x128[:], 1.0)
        # full [128,128] identity in bf16 (stationary base for diag(v_k))
        pidf = const.tile([128, 1], F32)
        nc.vector.tensor_copy(pidf[:], pidx_i[:])
        cidf = const.tile([128, 128], F32)
        nc.vector.tensor_copy(cidf[:], cidx_i[:])
        idf = const.tile([128, 128], F32)
        nc.vector.tensor_scalar(idf[:], cidf[:], pidf[:], 0.0, ALU.subtract, ALU.is_equal)
        idb = const.tile([128, 128], BF16)
        nc.vector.tensor_copy(idb[:], idf[:])

        def emit_extraction(psg):
            m = red.tile([128, 128], F32, tag="m")
            nc.vector.tensor_mul(m[:], psg[:], mask[:])
            psE = psx.tile([4, 128], F32, tag="psE")
            nc.tensor.matmul(psE[:], E4[:], m[:], start=True, stop=True)
            # S44[k, l] = sum_c psE[k, 4c+l]: strided-innermost reduce from PSUM
            S44 = red.tile([4, 4], F32, tag="S44")
            nc.vector.tensor_reduce(S44[:], _v(psE[:], [[1, 4], [4, 32]]),
                                    AXL.X, ALU.add)
            # spread S [4,4] -> [4,16] rows: Fm16[p,(j,l)] = S[p,l] * (p==j)
            Fm16 = red.tile([4, 16], F32, tag="Fm16")
            s_b = _v(S44[:], [[0, 4], [1, 4]])       # [4, j(bcast), l]
            nc.vector.tensor_tensor(Fm16[:].rearrange("p (j l) -> p j l", j=4),
                                    s_b, FM[0:4, :].rearrange("p (j l) -> p j l", j=4),
                                    ALU.mult)
            psS = psx.tile([128, 16], F32, tag="psS")
            nc.tensor.matmul(psS[:], ones4x128[:], Fm16[:], start=True, stop=True)
            return psS

        def emit_eig(b, psS):
            base = 24 * b
            negmu = auxp[:, base:base + 4]
            mmf = auxp[:, base + 4:base + 20]
            evec = auxp[:, 24 * BPC:24 * BPC + 4]
            covf = eig.tile([128, 16], F32, tag="covf")
            nc.vector.scalar_tensor_tensor(covf[:], psS[:], 1.0 / NROWS, mmf,
                                           ALU.mult, ALU.subtract)
            # -tr/4 via accum over covf * (-I/4)
            dgj = eig.tile([128, 16], F32, tag="dgj")
            trqn = eig.tile([128, 1], F32, tag="trqn")
            nc.vector.scalar_tensor_tensor(dgj[:], covf[:], 1.0, dm16qn[:],
                                           ALU.mult, ALU.mult, accum_out=trqn[:])
            B0 = eig.tile([128, 16], F32, tag="B0")
            nc.vector.scalar_tensor_tensor(B0[:], dm16[:], trqn[:], covf[:],
                                           ALU.mult, ALU.add)
            # shift s = 2*max|B0| (>= spectral bound margin for traceless 4x4)
            mx = eig.tile([128, 1], F32, tag="mx")
            nc.vector.tensor_reduce(mx[:], B0[:], AXL.X, ALU.max)
            mn = eig.tile([128, 1], F32, tag="mn")
            nc.vector.tensor_reduce(mn[:], B0[:], AXL.X, ALU.min)
            nc.vector.tensor_scalar(mx[:], mx[:], 2.0, None, ALU.mult)
            nc.vector.tensor_scalar(mn[:], mn[:], -2.0, None, ALU.mult)
            sft = eig.tile([128, 1], F32, tag="sft")
            nc.vector.tensor_tensor(sft[:], mx[:], mn[:], ALU.max)
            Bc = eig.tile([128, 16], F32, tag="Bc")
            nc.vector.scalar_tensor_tensor(Bc[:], dm16[:], sft[:], B0[:],
                                           ALU.mult, ALU.add)  # B0 + s I
            srec = eig.tile([128, 1], F32, tag="srec")
            nc.vector.reciprocal(srec[:], sft[:])
            nc.vector.tensor_scalar(Bc[:], Bc[:], srec[:], 0.5, ALU.mult, ALU.mult)
            # squarings, ping-pong; single renorm by 1/||C||_F^2 at it==2
            prod = eig.tile([128, 64], F32, tag="prod")
            Cc = eig.tile([128, 16], F32, tag="Cc")
            sqt = eig.tile([128, 16], F32, tag="sqt")
            cur, nxt = Bc, Cc
            for it in range(NSQ):
                b_ik = _v(cur[:], [[4, 4], [0, 4], [1, 4]])  # [p,i,j,k]=B[4i+k]
                b_kj = _v(cur[:], [[0, 4], [1, 4], [4, 4]])  # [p,i,j,k]=B[4k+j]
                nc.vector.tensor_tensor(
                    prod[:].rearrange("p (i j k) -> p i j k", i=4, j=4),
                    b_ik, b_kj, ALU.mult)
                nc.vector.tensor_reduce(
                    nxt[:].rearrange("p (i j) -> p i j", i=4),
                    prod[:].rearrange("p (i j k) -> p i j k", i=4, j=4),
                    AXL.X, ALU.add)
                if it == 2:
                    nrm2 = eig.tile([128, 1], F32, tag="nrm2")
                    nc.vector.scalar_tensor_tensor(
                        sqt[:], nxt[:], 1.0, nxt[:], ALU.mult, ALU.mult,
                        accum_out=nrm2[:])
                    nc.vector.reciprocal(nrm2[:], nrm2[:])
                    nc.vector.tensor_scalar(nxt[:], nxt[:], nrm2[:], None, ALU.mult)
                cur, nxt = nxt, cur
            # v = B @ e  (replicated): v_rep[p, i] = sum_j B[4i+j] e[j]
            vprod = eig.tile([128, 16], F32, tag="vprod")
            nc.vector.tensor_tensor(
                vprod[:].rearrange("p (i j) -> p i j", i=4),
                _v(cur[:], [[4, 4], [1, 4]]), _v(evec, [[0, 4], [1, 4]]), ALU.mult)
            v_rep = eig.tile([128, 4], F32, tag="v_rep")
            nc.vector.tensor_reduce(
                v_rep[:].rearrange("p (i u) -> p i u", i=4),
                vprod[:].rearrange("p (i j) -> p i j", i=4), AXL.X, ALU.add)
            # negc = v . (-mu) via accum
            mvp = eig.tile([128, 4], F32, tag="mvp")
            negc = eig.tile([128, 1], F32, tag="negc")
            nc.vector.scalar_tensor_tensor(mvp[:], v_rep[:], 1.0, negmu,
                                           ALU.mult, ALU.mult, accum_out=negc[:])
            # stats out: [1, 20] = Sflat | v_dev (DMA issued from gpsimd so
            # the sync queue's load triggers never wait on the eig chain)
            stt = eig.tile([1, 20], F32, tag="stt")
            nc.vector.tensor_copy(stt[:, 0:16], psS[0:1, :])
            nc.vector.tensor_copy(stt[:, 16:20], v_rep[0:1, :])
            nc.gpsimd.dma_start(bass.AP(st, b * 20, [[20, 1], [1, 20]]), stt[:])
            # stationary diag(v_k) bf16 matrices for the PE projection
            W4 = []
            for k in range(4):
                Wk = wpool.tile([128, 128], BF16, tag=f"W{k}", name=f"W_{b}_{k}")
                nc.vector.tensor_scalar(Wk[:], idb[:], v_rep[:, k:k + 1], None,
                                        ALU.mult)
                W4.append(Wk)
            return v_rep, negc, W4

        def emit_proj_chunk(pb, ci, t, W4, pnegc, stg):
            # PE projection of one chunk: per PSUM bank, 4 accumulating
            # matmuls with stationary diag(v_k), moving = strided channel-k
            # view; one ACT op applies -c and writes bf16; gpsimd DMAs out.
            half, q = divmod(ci, 2)
            for hf in range(2):
                ps = psv.tile([128, 512], F32, tag="sv",
                              name=f"sv_{pb}_{ci}_{hf}")
                for k in range(4):
                    rhs = bass.AP(t[:].tensor,
                                  t[:].offset + hf * 2048 + k,
                                  [list(t[:].ap[0])] + [[4, 512]])
                    nc.tensor.matmul(ps[:], W4[k][:], rhs,
                                     start=(k == 0), stop=(k == 3))
                nc.scalar.activation(
                    stg[:, ci * 1024 + hf * 512:ci * 1024 + (hf + 1) * 512],
                    ps[:], AF.Identity, bias=pnegc[:], scale=1.0)
            nc.gpsimd.dma_start(
                bass.AP(y, pb * OUT_SAMPLE + q * 2048 + half * 256,
                        [[4096, 128], [512, 4], [1, 256]]),
                _v(stg[:], [[256, 4], [1, 256]], extra_off=ci * 1024))

        def emit_proj_chunk_elem(pb, ci, t, pv_rep, pnegc, stg):
            # elementwise projection of one chunk (ACT/DVE/GP) — used in the
            # epilogue where the PE is the only saturated engine
            half, q = divmod(ci, 2)
            tview = t[:].rearrange("p (pixh jg k) -> p pixh jg k", pixh=4, k=4)
            sv = _v(stg[:], [[256, 4], [1, 256]], extra_off=ci * 1024)
            svf = stg[:, ci * 1024:(ci + 1) * 1024]
            pl2 = planes.tile([128, 1024], BF16, tag="pl2", name=f"pl2_{pb}_{ci}")
            nc.scalar.activation(pl2[:].rearrange("p (pixh jg) -> p pixh jg", pixh=4),
                                 tview[:, :, :, 2], AF.Identity,
                                 bias=0.0, scale=pv_rep[:, 2:3])
            pl3 = planes.tile([128, 1024], BF16, tag="pl3", name=f"pl3_{pb}_{ci}")
            nc.scalar.activation(pl3[:].rearrange("p (pixh jg) -> p pixh jg", pixh=4),
                                 tview[:, :, :, 3], AF.Identity,
                                 bias=pnegc[:], scale=pv_rep[:, 3:4])
            nc.vector.scalar_tensor_tensor(
                sv, tview[:, :, :, 0], pv_rep[:, 0:1],
                pl2[:].rearrange("p (pixh jg) -> p pixh jg", pixh=4),
                ALU.mult, ALU.add)
            nc.vector.scalar_tensor_tensor(
                sv, tview[:, :, :, 1], pv_rep[:, 1:2], sv, ALU.mult, ALU.add)
            nc.gpsimd.tensor_tensor(svf, pl3[:], svf, ALU.add)
            nc.gpsimd.dma_start(
                bass.AP(y, pb * OUT_SAMPLE + q * 2048 + half * 256,
                        [[4096, 128], [512, 4], [1, 256]]),
                _v(stg[:], [[256, 4], [1, 256]], extra_off=ci * 1024))

        # Lag-2 pipeline.
        states = {}
        vstate = {}
        for b in range(BPC):
            # ---------- loads: 8 half-chunk staging tiles; triggers split
            # across two queues to halve head-of-line blocking ----------
            ftiles = []
            for h in range(8):
                ci, h2 = divmod(h, 2)
                half, q = divmod(ci, 2)
                tf = stage.tile([128, 2048], F32, tag="stagef", name=f"tf_{b}_{h}")
                nc.sync.dma_start(tf[:], _in_dram_ap(x, b, half, q, h2))
                ftiles.append(tf)
            ctiles = [
                chunks.tile([128, 4096], BF16, tag="chunk", name=f"t_{b}_{ci}")
                for ci in range(4)
            ]
            prev2 = states.pop(b - 2, None)
            if prev2 is not None:
                _, pnegc, pW4 = vstate[b - 2]
                pstg = stag.tile([128, 4096], BF16, tag="stg", name=f"stg_{b-2}")
            psg = psum.tile([128, 128], F32, tag="psg")

            def cast_h(h):
                ci, h2 = divmod(h, 2)
                nc.vector.tensor_copy(
                    ctiles[ci][:, h2 * 2048:(h2 + 1) * 2048], ftiles[h][:])

            def gram_c(ci):
                for j in range(32):
                    lhs = ctiles[ci][:, j * 128:(j + 1) * 128]
                    nc.tensor.matmul(psg[:], lhs, lhs,
                                     start=(ci == 0 and j == 0),
                                     stop=(ci == 3 and j == 31))

            # Interleaved emission: keep the PE continuously fed — proj
            # chunks of b-2 alternate with extraction matmuls of b-1 and
            # gram chunks of b; DVE casts and eig weave between.
            if prev2 is not None:
                emit_proj_chunk(b - 2, 0, prev2[0], pW4, pnegc, pstg)
            cast_h(0); cast_h(1)
            psS = None
            if b - 1 in states:
                psS = emit_extraction(vstate.pop(("psg", b - 1)))
            if prev2 is not None:
                emit_proj_chunk(b - 2, 1, prev2[1], pW4, pnegc, pstg)
            cast_h(2); cast_h(3)
            gram_c(0)
            if prev2 is not None:
                emit_proj_chunk(b - 2, 2, prev2[2], pW4, pnegc, pstg)
            cast_h(4); cast_h(5)
            gram_c(1)
            if psS is not None:
                vstate[b - 1] = emit_eig(b - 1, psS)
            if prev2 is not None:
                emit_proj_chunk(b - 2, 3, prev2[3], pW4, pnegc, pstg)
            cast_h(6); cast_h(7)
            gram_c(2)
            gram_c(3)
            states[b] = ctiles
            vstate[("psg", b)] = psg
        # ---------- epilogue: PE + elementwise engines share the drain ----------
        b = BPC
        pv_rep, pnegc, pW4 = vstate[b - 2]
        pstg = stag.tile([128, 4096], BF16, tag="stg", name=f"stg_{b-2}")
        prev2 = states.pop(b - 2)
        psS = emit_extraction(vstate.pop(("psg", b - 1)))
        emit_proj_chunk_elem(b - 2, 3, prev2[3], pv_rep, pnegc, pstg)
        for ci in range(3):
            emit_proj_chunk(b - 2, ci, prev2[ci], pW4, pnegc, pstg)
        vstate[b - 1] = emit_eig(b - 1, psS)
        pv_rep, pnegc, pW4 = vstate[b - 1]
        pstg = stag.tile([128, 4096], BF16, tag="stg", name=f"stg_{b-1}")
        prev1 = states.pop(b - 1)
        emit_proj_chunk_elem(b - 1, 2, prev1[2], pv_rep, pnegc, pstg)
        emit_proj_chunk_elem(b - 1, 3, prev1[3], pv_rep, pnegc, pstg)
        for ci in range(2):
            emit_proj_chunk(b - 1, ci, prev1[ci], pW4, pnegc, pstg)
    nc.compile()
    return nc


_CACHE = {}


def _get(name, builder):
    if name not in _CACHE:
        _CACHE[name] = builder()
    return _CACHE[name]


def make_aux(mean):
    """mean: [BPC, 4] float -> aux array [128, AUXW]."""
    auxv = np.zeros((128, AUXW), np.float32)
    for b in range(BPC):
        base = 24 * b
        auxv[:, base:base + 4] = -mean[b].astype(np.float32)
        auxv[:, base + 4:base + 20] = np.outer(
            mean[b], mean[b]).astype(np.float32).reshape(16)
    auxv[:, 24 * BPC:24 * BPC + 4] = np.asarray(EVEC, np.float32)
    return auxv


def kernel(inputs: np.ndarray) -> np.ndarray:
    xx = np.ascontiguousarray(np.asarray(inputs, dtype=np.float32))
    assert xx.shape == (B, H, W, C), xx.shape
    xf = xx.reshape(N_CORES, BPC * SAMPLE)
    cores = list(range(N_CORES))
    mean = xx.reshape(B, NROWS, 4).mean(axis=1, dtype=np.float64)  # [B, 4]

    nc = _get("fused", _build_fused)
    in_maps = [
        {"x": xf[c], "aux": make_aux(mean[c * BPC:(c + 1) * BPC])} for c in cores
    ]
    r = run_bass_kernel_spmd(nc, in_maps, cores)
    stats = np.stack([r.results[c]["stats"] for c in cores]).reshape(B, 20)
    yv = np.stack(
        [np.asarray(r.results[c]["y"]).astype(np.float32) for c in cores]
    ).reshape(B, OUT_SAMPLE)

    import ml_dtypes
    S = stats[:, 0:16].reshape(B, 4, 4).astype(np.float64)
    # device projects with bf16-rounded v (diag stationaries); mirror that
    v_dev = stats[:, 16:20].astype(ml_dtypes.bfloat16).astype(np.float64)
    cov = (S / NROWS - np.einsum("bi,bj->bij", mean, mean)).astype(np.float32)

    import jax
    import jax.numpy as jnp
    with jax.default_device(jax.devices("cpu")[0]):
        _, vecs = jnp.linalg.eigh(jnp.asarray(cov))
    v_ref = np.asarray(vecs)[:, :, -1].astype(np.float64)

    dot = (v_ref * v_dev).sum(1)
    scale = np.sign(dot) / np.linalg.norm(v_dev, axis=1)
    yv = (yv * scale[:, None]).astype(np.float32)
    return yv.reshape(B, H // 2, W // 2, C)
